# revision 12
# baseline (speedup 1.0000x reference)
"""LorentzKG scoring kernel for 8 Trainium2 NeuronCores.

Strategy
--------
Data-parallel over the triple batch (131072 triples per core). The host does
index-based data movement only: it packs per-entity rows [emb(33)|bias] and a
per-relation table of precomputed coefficients (all cos/sin/cosh/sinh over the
1000 relations folds into 68 floats per relation), then gathers per-triple
feature rows with np.take. Each core streams its rows sequentially at full HBM
bandwidth and performs every per-triple floating-point op on device:

  DVE    : rotation (pre-scaled by cosh(vn)), boost substitution, res_sp,
           dot-product multiplies, the two 32-wide reductions, score chain
  ACT    : Square / Sqrt / Ln transcendentals
  GPSIMD : one 32-wide elementwise multiply (res_sp * t_sp)
  SP     : all stream DMAs (HWDGE), double-buffered

Math (reference refactored):
  ra = c*a - s*b ; rb = s*a + c*b          (c,s = cos/sin(rot[:16]))
  nx1 = c0*ra0 + s0*x0                     (c0,s0 = cosh/sinh(clip(boost0)))
  ns  = [nx1, ra1..15, rb0..15]
  res = cvn*ns + w                         (cvn = cosh(vn), w = sinh(vn)/vn*0.1*trans)
  time = sqrt(1+|res|^2)
  ic  = max(time*t0 - res.t_sp, 1+1e-6)
  score = -(log(ic + sqrt(ic^2-1)))^2 + bh + bt
cvn folds into the packed (C,S) coefficients so res = rot_result + w directly.
"""
import numpy as np

import concourse.bass as bass
import concourse.mybir as mybir
from concourse.bass_utils import run_bass_kernel_spmd

NE = 1_000_000
NR = 1000
D = 32
B = 1_048_576
NCORES = 8
BCORE = B // NCORES          # 131072
P = 128
K = 64                       # triples per partition per chunk
CHUNK = P * K                # 8192
NCH = BCORE // CHUNK         # 16
HW = 34                      # head/tail row width
RW = 68                      # relation row width

TRACE = False
USE_GPSIMD_PD = False
DBG_OUT = None
LAST_EXEC_NS = None

_NC_CACHE = []

F32 = mybir.dt.float32
MUL = mybir.AluOpType.mult
ADD = mybir.AluOpType.add
SUB = mybir.AluOpType.subtract


def _build_nc():
    nc = bass.Bass()
    h_in = nc.declare_dram_parameter("h", [BCORE, HW], F32, isOutput=False)
    t_in = nc.declare_dram_parameter("t", [BCORE, HW], F32, isOutput=False)
    r_in = nc.declare_dram_parameter("r", [BCORE, RW], F32, isOutput=False)
    out = nc.declare_dram_parameter("out", [BCORE], F32, isOutput=True)

    h_d = h_in[:].rearrange("(c p k) d -> c p (k d)", p=P, k=K)
    t_d = t_in[:].rearrange("(c p k) d -> c p (k d)", p=P, k=K)
    r_d = r_in[:].rearrange("(c p k) d -> c p (k d)", p=P, k=K)
    o_d = out[:].rearrange("(c p k) -> c p k", p=P, k=K)

    ctx_list = []

    def sb(name, width):
        cm = nc.sbuf_tensor([P, 2 * width], F32)
        t = cm.__enter__()
        ctx_list.append(cm)
        return t

    h_sb = sb("h", K * HW)
    t_sb = sb("t", K * HW)
    r_sb = sb("r", K * RW)
    p1_sb = sb("p1", K * 32)
    ns_sb = sb("ns", K * 32)
    sq_sb = sb("sq", K * 32)
    pd_sb = sb("pd", K * 32)
    smalls = {n: sb(n, K) for n in
              ["xs", "t0c", "r2", "rp1", "dot", "time", "u1", "ic",
               "qm", "s2", "u", "dd", "b1", "o"]}

    sems = {}
    for n in ["in_sem", "outst", "v_rs", "v_r2", "v_ic", "v_u", "v_done",
              "a_sq", "a_time", "a_s2", "a_dd", "g_pd"]:
        cm = nc.semaphore(n)
        sems[n] = cm.__enter__()
        ctx_list.append(cm)

    def hv(j):  # [p, k, d] view of slot
        s = j % 2
        return h_sb[:, s * K * HW:(s + 1) * K * HW].rearrange(
            "p (k d) -> p k d", d=HW)

    def tv(j):
        s = j % 2
        return t_sb[:, s * K * HW:(s + 1) * K * HW].rearrange(
            "p (k d) -> p k d", d=HW)

    def rv(j):
        s = j % 2
        return r_sb[:, s * K * RW:(s + 1) * K * RW].rearrange(
            "p (k d) -> p k d", d=RW)

    def w32(t, j):  # [p, k, 32] view of a K*32 buffer slot
        s = j % 2
        return t[:, s * K * 32:(s + 1) * K * 32].rearrange(
            "p (k d) -> p k d", d=32)

    def sm(n, j):
        s = j % 2
        return smalls[n][:, s * K:(s + 1) * K]

    blk_cm = nc.Block()
    blk = blk_cm.__enter__()

    @blk.sync
    def _(sync):
        for j in range(min(2, NCH)):
            sync.dma_start(out=hv(j), in_=h_d[j]).then_inc(sems["in_sem"], 16)
            sync.dma_start(out=tv(j), in_=t_d[j]).then_inc(sems["in_sem"], 16)
            sync.dma_start(out=rv(j), in_=r_d[j]).then_inc(sems["in_sem"], 16)
        for j in range(NCH):
            sync.wait_ge(sems["v_done"], j + 1)
            sync.dma_start(out=o_d[j], in_=sm("o", j)).then_inc(sems["outst"], 16)
            if j + 2 < NCH:
                sync.dma_start(out=hv(j + 2), in_=h_d[j + 2]).then_inc(sems["in_sem"], 16)
                sync.dma_start(out=tv(j + 2), in_=t_d[j + 2]).then_inc(sems["in_sem"], 16)
                sync.dma_start(out=rv(j + 2), in_=r_d[j + 2]).then_inc(sems["in_sem"], 16)

    @blk.vector
    def _(vector):
        tt = nc.vector.tensor_tensor
        for j in range(NCH):
            H, T, R = hv(j), tv(j), rv(j)
            P1, NS, SQ, PD = (w32(p1_sb, j), w32(ns_sb, j),
                              w32(sq_sb, j), w32(pd_sb, j))
            vector.wait_ge(sems["in_sem"], 48 * (j + 1))
            # rotation, pre-scaled by cvn (and slot0 by nothing extra)
            tt(out=P1[:, :, :], in0=R[:, :, 0:32], in1=H[:, :, 1:33], op=MUL)
            tt(out=NS[:, :, 0:16], in0=P1[:, :, 0:16], in1=P1[:, :, 16:32], op=SUB)
            tt(out=P1[:, :, 0:16], in0=R[:, :, 16:32], in1=H[:, :, 1:17], op=MUL)
            tt(out=P1[:, :, 16:32], in0=R[:, :, 0:16], in1=H[:, :, 17:33], op=MUL)
            tt(out=NS[:, :, 16:32], in0=P1[:, :, 0:16], in1=P1[:, :, 16:32], op=ADD)
            # boost: ns0 = c0*(cvn*ra0) + x0*(cvn*s0)
            tt(out=sm("xs", j), in0=H[:, :, 0], in1=R[:, :, 64], op=MUL)
            tt(out=sm("t0c", j), in0=NS[:, :, 0], in1=R[:, :, 65], op=MUL)
            tt(out=NS[:, :, 0], in0=sm("t0c", j), in1=sm("xs", j), op=ADD)
            # res_sp = ns + w  (cvn already folded in)
            tt(out=NS[:, :, :], in0=NS[:, :, :], in1=R[:, :, 32:64], op=ADD)
            vector.drain()
            vector.sem_inc(sems["v_rs"], 1)
            if not USE_GPSIMD_PD:
                tt(out=PD[:, :, :], in0=NS[:, :, :], in1=T[:, :, 1:33], op=MUL)
            vector.wait_ge(sems["a_sq"], j + 1)
            nc.vector.reduce_sum(out=sm("r2", j), in_=SQ[:, :, :],
                                 axis=mybir.AxisListType.X)
            nc.vector.reduce_sum(out=sm("dot", j), in_=PD[:, :, :],
                                 axis=mybir.AxisListType.X)
            # time = sqrt(1+r2) via degree-4 Horner (r2 <= 0.02, rel err 6e-11)
            # drains: tensor_scalar writes are not RAW-safe for the next op
            x = sm("r2", j)
            nc.vector.tensor_scalar(sm("xs", j), x, -0.0390625, 0.0625,
                                    MUL, ADD)
            vector.drain()
            tt(out=sm("t0c", j), in0=sm("xs", j), in1=x, op=MUL)
            nc.vector.tensor_scalar_add(sm("t0c", j), sm("t0c", j), -0.125)
            vector.drain()
            tt(out=sm("xs", j), in0=sm("t0c", j), in1=x, op=MUL)
            nc.vector.tensor_scalar_add(sm("xs", j), sm("xs", j), 0.5)
            vector.drain()
            tt(out=sm("time", j), in0=sm("xs", j), in1=x, op=MUL)
            nc.vector.tensor_scalar_add(sm("time", j), sm("time", j), 1.0)
            vector.drain()
            tt(out=sm("u1", j), in0=sm("time", j), in1=T[:, :, 0], op=MUL)
            tt(out=sm("u1", j), in0=sm("u1", j), in1=sm("dot", j), op=SUB)
            nc.vector.tensor_scalar_max(sm("ic", j), sm("u1", j), 1.0 + 1e-6)
            vector.drain()
            nc.vector.tensor_scalar_add(sm("xs", j), sm("ic", j), -1.0)
            nc.vector.tensor_scalar_add(sm("t0c", j), sm("ic", j), 1.0)
            vector.drain()
            tt(out=sm("qm", j), in0=sm("xs", j), in1=sm("t0c", j), op=MUL)
            vector.drain()
            vector.sem_inc(sems["v_ic"], 1)
            vector.wait_ge(sems["a_s2"], j + 1)
            tt(out=sm("u", j), in0=sm("ic", j), in1=sm("s2", j), op=ADD)
            vector.drain()
            vector.sem_inc(sems["v_u"], 1)
            tt(out=sm("b1", j), in0=H[:, :, 33], in1=T[:, :, 33], op=ADD)
            if j >= 2:
                vector.wait_ge(sems["outst"], 16 * (j - 1))
            vector.wait_ge(sems["a_dd"], j + 1)
            if DBG_OUT is None:
                tt(out=sm("o", j), in0=sm("b1", j), in1=sm("dd", j), op=SUB)
            else:
                nc.vector.tensor_copy(out=sm("o", j), in_=sm(DBG_OUT, j))
            vector.drain()
            vector.sem_inc(sems["v_done"], 1)

    @blk.scalar
    def _(scalar):
        act = nc.scalar.activation
        AF = mybir.ActivationFunctionType
        for j in range(NCH):
            NS, SQ = w32(ns_sb, j), w32(sq_sb, j)
            scalar.wait_ge(sems["v_rs"], j + 1)
            act(out=SQ[:, :, :], in_=NS[:, :, :], func=AF.Square)
            scalar.drain()
            scalar.sem_inc(sems["a_sq"], 1)
            scalar.wait_ge(sems["v_ic"], j + 1)
            act(out=sm("s2", j), in_=sm("qm", j), func=AF.Sqrt)
            scalar.drain()
            scalar.sem_inc(sems["a_s2"], 1)
            scalar.wait_ge(sems["v_u"], j + 1)
            act(out=sm("dd", j), in_=sm("u", j), func=AF.Ln)
            act(out=sm("dd", j), in_=sm("dd", j), func=AF.Square)
            scalar.drain()
            scalar.sem_inc(sems["a_dd"], 1)


    blk_cm.__exit__(None, None, None)
    # keep sbuf/semaphore contexts open for the lifetime of nc
    nc._ctx_keepalive = ctx_list
    return nc


def _get_nc():
    if not _NC_CACHE:
        _NC_CACHE.append(_build_nc())
    return _NC_CACHE[0]


def _host_pack(heads, relations, tails, entity_emb, rel_boost_w, rel_rot_w,
               rel_trans_w, ent_bias_w):
    heads = np.asarray(heads).astype(np.int64)
    relations = np.asarray(relations).astype(np.int64)
    tails = np.asarray(tails).astype(np.int64)
    entity_emb = np.asarray(entity_emb, dtype=np.float32)
    ent_bias_w = np.asarray(ent_bias_w, dtype=np.float32)

    # per-relation precompute in float64, rounded to f32
    rot = np.asarray(rel_rot_w, dtype=np.float32).astype(np.float64)
    boost = np.asarray(rel_boost_w, dtype=np.float32).astype(np.float64)
    trans = np.asarray(rel_trans_w, dtype=np.float32).astype(np.float64)

    c = np.cos(rot[:, :16])
    s = np.sin(rot[:, :16])
    rap0 = np.clip(boost[:, 0], -2.0, 2.0)
    c0 = np.cosh(rap0)
    s0 = np.sinh(rap0)
    tv = 0.1 * trans
    vn = np.sqrt(np.clip(np.sum(tv * tv, axis=1), 1e-6, None))
    cvn = np.cosh(vn)
    w = (np.sinh(vn) / vn)[:, None] * tv

    rel_packed = np.zeros((NR, RW), dtype=np.float32)
    rel_packed[:, 0:16] = (cvn[:, None] * c).astype(np.float32)
    rel_packed[:, 16:32] = (cvn[:, None] * s).astype(np.float32)
    rel_packed[:, 32:64] = w.astype(np.float32)
    rel_packed[:, 64] = (cvn * s0).astype(np.float32)
    rel_packed[:, 65] = c0.astype(np.float32)

    ent_packed = np.concatenate([entity_emb, ent_bias_w], axis=1)  # [NE, 34]

    h_stream = ent_packed[heads]
    t_stream = ent_packed[tails]
    r_stream = rel_packed[relations]
    return h_stream, t_stream, r_stream


def kernel(heads, relations, tails, entity_emb, rel_boost_w, rel_rot_w,
           rel_trans_w, ent_bias_w):
    global LAST_EXEC_NS
    h_stream, t_stream, r_stream = _host_pack(
        heads, relations, tails, entity_emb, rel_boost_w, rel_rot_w,
        rel_trans_w, ent_bias_w)

    nc = _get_nc()
    in_maps = []
    for i in range(NCORES):
        sl = slice(i * BCORE, (i + 1) * BCORE)
        in_maps.append({"h": np.ascontiguousarray(h_stream[sl]),
                        "t": np.ascontiguousarray(t_stream[sl]),
                        "r": np.ascontiguousarray(r_stream[sl])})

    res = run_bass_kernel_spmd(nc, in_maps, core_ids=list(range(NCORES)),
                               trace=TRACE)
    LAST_EXEC_NS = res.exec_time_ns
    return np.concatenate([res.results[i]["out"] for i in range(NCORES)])


# revision 13
# speedup vs baseline: 1.2338x; 1.2338x over previous
"""LorentzKG scoring kernel for 8 Trainium2 NeuronCores.

Strategy
--------
Data-parallel over the triple batch (131072 triples per core). The host does
index-based data movement only: it packs per-entity rows [emb(33)|bias] and a
per-relation table of precomputed coefficients (all cos/sin/cosh/sinh over the
1000 relations folds into 68 floats per relation), then gathers per-triple
feature rows with np.take. Each core streams its rows sequentially at full HBM
bandwidth and performs every per-triple floating-point op on device:

  DVE    : rotation (pre-scaled by cosh(vn)), boost substitution, res_sp,
           dot-product multiplies, the two 32-wide reductions, score chain
  ACT    : Square / Sqrt / Ln transcendentals
  GPSIMD : one 32-wide elementwise multiply (res_sp * t_sp)
  SP     : all stream DMAs (HWDGE), double-buffered

Math (reference refactored):
  ra = c*a - s*b ; rb = s*a + c*b          (c,s = cos/sin(rot[:16]))
  nx1 = c0*ra0 + s0*x0                     (c0,s0 = cosh/sinh(clip(boost0)))
  ns  = [nx1, ra1..15, rb0..15]
  res = cvn*ns + w                         (cvn = cosh(vn), w = sinh(vn)/vn*0.1*trans)
  time = sqrt(1+|res|^2)
  ic  = max(time*t0 - res.t_sp, 1+1e-6)
  score = -(log(ic + sqrt(ic^2-1)))^2 + bh + bt
cvn folds into the packed (C,S) coefficients so res = rot_result + w directly.
"""
import numpy as np

import concourse.bass as bass
import concourse.mybir as mybir
from concourse.bass_utils import run_bass_kernel_spmd

NE = 1_000_000
NR = 1000
D = 32
B = 1_048_576
NCORES = 8
BCORE = B // NCORES          # 131072
P = 128
K = 64                       # triples per partition per chunk
CHUNK = P * K                # 8192
NCH = BCORE // CHUNK         # 16
HW = 34                      # head/tail row width
RW = 68                      # relation row width

TRACE = False
USE_GPSIMD_PD = False
DBG_OUT = None
LAST_EXEC_NS = None

_NC_CACHE = []

F32 = mybir.dt.float32
MUL = mybir.AluOpType.mult
ADD = mybir.AluOpType.add
SUB = mybir.AluOpType.subtract


def _build_nc():
    nc = bass.Bass()
    h_in = nc.declare_dram_parameter("h", [BCORE, HW], F32, isOutput=False)
    t_in = nc.declare_dram_parameter("t", [BCORE, HW], F32, isOutput=False)
    r_in = nc.declare_dram_parameter("r", [BCORE, RW], F32, isOutput=False)
    cst_in = nc.declare_dram_parameter("cst", [P, 8], F32, isOutput=False)
    out = nc.declare_dram_parameter("out", [BCORE], F32, isOutput=True)

    h_d = h_in[:].rearrange("(c p k) d -> c p (k d)", p=P, k=K)
    t_d = t_in[:].rearrange("(c p k) d -> c p (k d)", p=P, k=K)
    r_d = r_in[:].rearrange("(c p k) d -> c p (k d)", p=P, k=K)
    o_d = out[:].rearrange("(c p k) -> c p k", p=P, k=K)

    ctx_list = []

    def sb(name, width):
        cm = nc.sbuf_tensor([P, 2 * width], F32)
        t = cm.__enter__()
        ctx_list.append(cm)
        return t

    cst_sb = sb("cst", 8)[:, 0:8]
    h_sb = sb("h", K * HW)
    t_sb = sb("t", K * HW)
    r_sb = sb("r", K * RW)
    p1_sb = sb("p1", K * 32)
    ns_sb = sb("ns", K * 32)
    sq_sb = sb("sq", K * 32)
    pd_sb = sb("pd", K * 32)
    smalls = {n: sb(n, K) for n in
              ["xs", "t0c", "r2", "rp1", "dot", "time", "u1", "ic",
               "qm", "s2", "u", "dd", "b1", "o"]}

    sems = {}
    for n in ["in_sem", "outst", "v_rs", "v_ic", "v_u", "v_done",
              "a_sq", "a_s2", "a_dd", "c_sem"]:
        cm = nc.semaphore(n)
        sems[n] = cm.__enter__()
        ctx_list.append(cm)

    def hv(j):  # [p, k, d] view of slot
        s = j % 2
        return h_sb[:, s * K * HW:(s + 1) * K * HW].rearrange(
            "p (k d) -> p k d", d=HW)

    def tv(j):
        s = j % 2
        return t_sb[:, s * K * HW:(s + 1) * K * HW].rearrange(
            "p (k d) -> p k d", d=HW)

    def rv(j):
        s = j % 2
        return r_sb[:, s * K * RW:(s + 1) * K * RW].rearrange(
            "p (k d) -> p k d", d=RW)

    def w32(t, j):  # [p, k, 32] view of a K*32 buffer slot
        s = j % 2
        return t[:, s * K * 32:(s + 1) * K * 32].rearrange(
            "p (k d) -> p k d", d=32)

    def sm(n, j):
        s = j % 2
        return smalls[n][:, s * K:(s + 1) * K]

    blk_cm = nc.Block()
    blk = blk_cm.__enter__()

    @blk.sync
    def _(sync):
        sync.dma_start(out=cst_sb, in_=cst_in[:]).then_inc(sems["c_sem"], 16)
        for j in range(min(2, NCH)):
            sync.dma_start(out=hv(j), in_=h_d[j]).then_inc(sems["in_sem"], 16)
            sync.dma_start(out=tv(j), in_=t_d[j]).then_inc(sems["in_sem"], 16)
            sync.dma_start(out=rv(j), in_=r_d[j]).then_inc(sems["in_sem"], 16)
        for j in range(NCH):
            sync.wait_ge(sems["v_done"], j + 1)
            sync.dma_start(out=o_d[j], in_=sm("o", j)).then_inc(sems["outst"], 16)
            if j + 2 < NCH:
                sync.dma_start(out=hv(j + 2), in_=h_d[j + 2]).then_inc(sems["in_sem"], 16)
                sync.dma_start(out=tv(j + 2), in_=t_d[j + 2]).then_inc(sems["in_sem"], 16)
                sync.dma_start(out=rv(j + 2), in_=r_d[j + 2]).then_inc(sems["in_sem"], 16)

    @blk.vector
    def _(vector):
        tt = nc.vector.tensor_tensor
        def cb_(i):
            return cst_sb[:, i:i + 1].to_broadcast([P, K])
        vector.wait_ge(sems["c_sem"], 16)
        for j in range(NCH):
            H, T, R = hv(j), tv(j), rv(j)
            P1, NS, SQ, PD = (w32(p1_sb, j), w32(ns_sb, j),
                              w32(sq_sb, j), w32(pd_sb, j))
            vector.wait_ge(sems["in_sem"], 48 * (j + 1))
            # rotation, pre-scaled by cvn (and slot0 by nothing extra)
            tt(out=P1[:, :, :], in0=R[:, :, 0:32], in1=H[:, :, 1:33], op=MUL)
            tt(out=NS[:, :, 0:16], in0=P1[:, :, 0:16], in1=P1[:, :, 16:32], op=SUB)
            tt(out=P1[:, :, 0:16], in0=R[:, :, 16:32], in1=H[:, :, 1:17], op=MUL)
            tt(out=P1[:, :, 16:32], in0=R[:, :, 0:16], in1=H[:, :, 17:33], op=MUL)
            tt(out=NS[:, :, 16:32], in0=P1[:, :, 0:16], in1=P1[:, :, 16:32], op=ADD)
            # boost: ns0 = c0*(cvn*ra0) + x0*(cvn*s0)
            tt(out=sm("xs", j), in0=H[:, :, 0], in1=R[:, :, 64], op=MUL)
            tt(out=sm("t0c", j), in0=NS[:, :, 0], in1=R[:, :, 65], op=MUL)
            tt(out=NS[:, :, 0], in0=sm("t0c", j), in1=sm("xs", j), op=ADD)
            # res_sp = ns + w  (cvn already folded in)
            tt(out=NS[:, :, :], in0=NS[:, :, :], in1=R[:, :, 32:64], op=ADD)
            vector.drain()
            vector.sem_inc(sems["v_rs"], 1)
            if not USE_GPSIMD_PD:
                tt(out=PD[:, :, :], in0=NS[:, :, :], in1=T[:, :, 1:33], op=MUL)
            vector.wait_ge(sems["a_sq"], j + 1)
            nc.vector.reduce_sum(out=sm("r2", j), in_=SQ[:, :, :],
                                 axis=mybir.AxisListType.X)
            nc.vector.reduce_sum(out=sm("dot", j), in_=PD[:, :, :],
                                 axis=mybir.AxisListType.X)
            # time = sqrt(1+r2) via degree-4 Horner (r2 <= 0.02, rel err 6e-11)
            # tensor_tensor with broadcast const columns only (RAW-safe; the
            # tensor_scalar family has lagging write visibility on this HW)
            x = sm("r2", j)
            tt(out=sm("xs", j), in0=x, in1=cb_(0), op=MUL)
            tt(out=sm("xs", j), in0=sm("xs", j), in1=cb_(1), op=ADD)
            tt(out=sm("t0c", j), in0=sm("xs", j), in1=x, op=MUL)
            tt(out=sm("t0c", j), in0=sm("t0c", j), in1=cb_(2), op=ADD)
            tt(out=sm("xs", j), in0=sm("t0c", j), in1=x, op=MUL)
            tt(out=sm("xs", j), in0=sm("xs", j), in1=cb_(3), op=ADD)
            tt(out=sm("time", j), in0=sm("xs", j), in1=x, op=MUL)
            tt(out=sm("time", j), in0=sm("time", j), in1=cb_(4), op=ADD)
            tt(out=sm("u1", j), in0=sm("time", j), in1=T[:, :, 0], op=MUL)
            tt(out=sm("u1", j), in0=sm("u1", j), in1=sm("dot", j), op=SUB)
            tt(out=sm("ic", j), in0=sm("u1", j), in1=cb_(5),
               op=mybir.AluOpType.max)
            tt(out=sm("xs", j), in0=sm("ic", j), in1=cb_(4), op=SUB)
            tt(out=sm("t0c", j), in0=sm("ic", j), in1=cb_(4), op=ADD)
            tt(out=sm("qm", j), in0=sm("xs", j), in1=sm("t0c", j), op=MUL)
            vector.drain()
            vector.sem_inc(sems["v_ic"], 1)
            vector.wait_ge(sems["a_s2"], j + 1)
            tt(out=sm("u", j), in0=sm("ic", j), in1=sm("s2", j), op=ADD)
            vector.drain()
            vector.sem_inc(sems["v_u"], 1)
            tt(out=sm("b1", j), in0=H[:, :, 33], in1=T[:, :, 33], op=ADD)
            if j >= 2:
                vector.wait_ge(sems["outst"], 16 * (j - 1))
            vector.wait_ge(sems["a_dd"], j + 1)
            if DBG_OUT is None:
                tt(out=sm("o", j), in0=sm("b1", j), in1=sm("dd", j), op=SUB)
            else:
                nc.vector.tensor_copy(out=sm("o", j), in_=sm(DBG_OUT, j))
            vector.drain()
            vector.sem_inc(sems["v_done"], 1)

    @blk.scalar
    def _(scalar):
        act = nc.scalar.activation
        AF = mybir.ActivationFunctionType
        for j in range(NCH):
            NS, SQ = w32(ns_sb, j), w32(sq_sb, j)
            scalar.wait_ge(sems["v_rs"], j + 1)
            act(out=SQ[:, :, :], in_=NS[:, :, :], func=AF.Square)
            scalar.drain()
            scalar.sem_inc(sems["a_sq"], 1)
            scalar.wait_ge(sems["v_ic"], j + 1)
            act(out=sm("s2", j), in_=sm("qm", j), func=AF.Sqrt)
            scalar.drain()
            scalar.sem_inc(sems["a_s2"], 1)
            scalar.wait_ge(sems["v_u"], j + 1)
            act(out=sm("dd", j), in_=sm("u", j), func=AF.Ln)
            act(out=sm("dd", j), in_=sm("dd", j), func=AF.Square)
            scalar.drain()
            scalar.sem_inc(sems["a_dd"], 1)


    blk_cm.__exit__(None, None, None)
    # keep sbuf/semaphore contexts open for the lifetime of nc
    nc._ctx_keepalive = ctx_list
    return nc


def _get_nc():
    if not _NC_CACHE:
        _NC_CACHE.append(_build_nc())
    return _NC_CACHE[0]


def _host_pack(heads, relations, tails, entity_emb, rel_boost_w, rel_rot_w,
               rel_trans_w, ent_bias_w):
    heads = np.asarray(heads).astype(np.int64)
    relations = np.asarray(relations).astype(np.int64)
    tails = np.asarray(tails).astype(np.int64)
    entity_emb = np.asarray(entity_emb, dtype=np.float32)
    ent_bias_w = np.asarray(ent_bias_w, dtype=np.float32)

    # per-relation precompute in float64, rounded to f32
    rot = np.asarray(rel_rot_w, dtype=np.float32).astype(np.float64)
    boost = np.asarray(rel_boost_w, dtype=np.float32).astype(np.float64)
    trans = np.asarray(rel_trans_w, dtype=np.float32).astype(np.float64)

    c = np.cos(rot[:, :16])
    s = np.sin(rot[:, :16])
    rap0 = np.clip(boost[:, 0], -2.0, 2.0)
    c0 = np.cosh(rap0)
    s0 = np.sinh(rap0)
    tv = 0.1 * trans
    vn = np.sqrt(np.clip(np.sum(tv * tv, axis=1), 1e-6, None))
    cvn = np.cosh(vn)
    w = (np.sinh(vn) / vn)[:, None] * tv

    rel_packed = np.zeros((NR, RW), dtype=np.float32)
    rel_packed[:, 0:16] = (cvn[:, None] * c).astype(np.float32)
    rel_packed[:, 16:32] = (cvn[:, None] * s).astype(np.float32)
    rel_packed[:, 32:64] = w.astype(np.float32)
    rel_packed[:, 64] = (cvn * s0).astype(np.float32)
    rel_packed[:, 65] = c0.astype(np.float32)

    ent_packed = np.concatenate([entity_emb, ent_bias_w], axis=1)  # [NE, 34]

    h_stream = ent_packed[heads]
    t_stream = ent_packed[tails]
    r_stream = rel_packed[relations]
    return h_stream, t_stream, r_stream


def kernel(heads, relations, tails, entity_emb, rel_boost_w, rel_rot_w,
           rel_trans_w, ent_bias_w):
    global LAST_EXEC_NS
    h_stream, t_stream, r_stream = _host_pack(
        heads, relations, tails, entity_emb, rel_boost_w, rel_rot_w,
        rel_trans_w, ent_bias_w)

    nc = _get_nc()
    cst = np.zeros((P, 8), dtype=np.float32)
    cst[:, 0] = -0.0390625
    cst[:, 1] = 0.0625
    cst[:, 2] = -0.125
    cst[:, 3] = 0.5
    cst[:, 4] = 1.0
    cst[:, 5] = 1.0 + 1e-6
    in_maps = []
    for i in range(NCORES):
        sl = slice(i * BCORE, (i + 1) * BCORE)
        in_maps.append({"h": np.ascontiguousarray(h_stream[sl]),
                        "t": np.ascontiguousarray(t_stream[sl]),
                        "r": np.ascontiguousarray(r_stream[sl]),
                        "cst": cst})

    res = run_bass_kernel_spmd(nc, in_maps, core_ids=list(range(NCORES)),
                               trace=TRACE)
    LAST_EXEC_NS = res.exec_time_ns
    return np.concatenate([res.results[i]["out"] for i in range(NCORES)])


# revision 15
# speedup vs baseline: 1.4064x; 1.1399x over previous
"""LorentzKG scoring kernel for 8 Trainium2 NeuronCores. v2: 4-stage softpipe.

See kernel.py docstring for the math. Engine schedule per DVE iteration j:
  big(j):    rotation/boost/res (reads H,T,R slots) -> v_big frees the slots
  reduce(j): r2/dot reductions (after ACT's square, a_sq)
  tailA(j-1): sqrt-poly + ic + qm            -> v_ic
  tailB1(j-2): u = ic + s2 (after ACT sqrt)  -> v_u
  tailB2(j-3): score = b1 - dd (after ACT ln/square) -> v_done
ACT per iteration j: square(j); s2(j-1); ln+dd(j-2).
All cross-engine handoffs are one-plus iterations apart -> no stalls.
tensor_scalar-family ops are avoided on the data path (their SBUF writes
are not reliably visible to the next consumer on this HW); constants come
from a small broadcast table instead.
"""
import numpy as np

import concourse.bass as bass
import concourse.mybir as mybir
from concourse.bass_utils import run_bass_kernel_spmd

NE = 1_000_000
NR = 1000
D = 32
B = 1_048_576
NCORES = 8
BCORE = B // NCORES          # 131072
P = 128
K = 64                       # triples per partition per chunk
CHUNK = P * K                # 8192
NCH = BCORE // CHUNK         # 16
HW = 34                      # head/tail row width
RW = 68                      # relation row width

TRACE = False
LAST_EXEC_NS = None
DBG_OUT = None

_NC_CACHE = []

F32 = mybir.dt.float32
MUL = mybir.AluOpType.mult
ADD = mybir.AluOpType.add
SUB = mybir.AluOpType.subtract
MAX = mybir.AluOpType.max

SM4 = ["tt0", "b1", "r2", "dot", "time", "xs", "t0c", "u1", "ic", "qm",
       "s2", "u", "dd"]


def _build_nc():
    nc = bass.Bass()
    h_in = nc.declare_dram_parameter("h", [BCORE, HW], F32, isOutput=False)
    t_in = nc.declare_dram_parameter("t", [BCORE, HW], F32, isOutput=False)
    r_in = nc.declare_dram_parameter("r", [BCORE, RW], F32, isOutput=False)
    cst_in = nc.declare_dram_parameter("cst", [P, 8], F32, isOutput=False)
    out = nc.declare_dram_parameter("out", [BCORE], F32, isOutput=True)

    h_d = h_in[:].rearrange("(c p k) d -> c p (k d)", p=P, k=K)
    t_d = t_in[:].rearrange("(c p k) d -> c p (k d)", p=P, k=K)
    r_d = r_in[:].rearrange("(c p k) d -> c p (k d)", p=P, k=K)
    o_d = out[:].rearrange("(c p k) -> c p k", p=P, k=K)

    ctx_list = []

    def sb(width):
        cm = nc.sbuf_tensor([P, width], F32)
        t = cm.__enter__()
        ctx_list.append(cm)
        return t

    cst_sb = sb(8)
    h_sb = sb(2 * K * HW)
    t_sb = sb(2 * K * HW)
    r_sb = sb(2 * K * RW)
    ns_sb = sb(2 * K * 32)
    sq_sb = sb(2 * K * 32)
    pp_sb = sb(2 * K * 32)       # P1 during rotation, then PD
    o_sb = sb(2 * K)
    smalls = {n: sb(4 * K) for n in SM4}

    sems = {}
    for n in ["in_sem", "outst", "c_sem", "v_big", "v_rs", "v_ic", "v_u",
              "v_done", "a_sq", "a_s2", "a_dd"]:
        cm = nc.semaphore(n)
        sems[n] = cm.__enter__()
        ctx_list.append(cm)

    def view(t, j, width, d):
        s = j % 2
        return t[:, s * K * width:(s + 1) * K * width].rearrange(
            "p (k d) -> p k d", d=d)

    def hv(j):
        return view(h_sb, j, HW, HW)

    def tv(j):
        return view(t_sb, j, HW, HW)

    def rv(j):
        return view(r_sb, j, RW, RW)

    def nsv(j):
        return view(ns_sb, j, 32, 32)

    def sqv(j):
        return view(sq_sb, j, 32, 32)

    def ppv(j):
        return view(pp_sb, j, 32, 32)

    def ov(j):
        s = j % 2
        return o_sb[:, s * K:(s + 1) * K]

    def sm(n, j):
        s = j % 4
        return smalls[n][:, s * K:(s + 1) * K]

    blk_cm = nc.Block()
    blk = blk_cm.__enter__()

    @blk.sync
    def _(sync):
        sync.dma_start(out=cst_sb[:, 0:8], in_=cst_in[:]).then_inc(
            sems["c_sem"], 16)
        for j in range(min(2, NCH)):
            sync.dma_start(out=hv(j), in_=h_d[j]).then_inc(sems["in_sem"], 16)
            sync.dma_start(out=tv(j), in_=t_d[j]).then_inc(sems["in_sem"], 16)
            sync.dma_start(out=rv(j), in_=r_d[j]).then_inc(sems["in_sem"], 16)
        for j in range(NCH):
            if j + 2 < NCH:
                sync.wait_ge(sems["v_big"], j + 1)
                sync.dma_start(out=hv(j + 2), in_=h_d[j + 2]).then_inc(
                    sems["in_sem"], 16)
                sync.dma_start(out=tv(j + 2), in_=t_d[j + 2]).then_inc(
                    sems["in_sem"], 16)
                sync.dma_start(out=rv(j + 2), in_=r_d[j + 2]).then_inc(
                    sems["in_sem"], 16)
            if j >= 3:
                sync.wait_ge(sems["v_done"], j - 2)
                sync.dma_start(out=o_d[j - 3], in_=ov(j - 3)).then_inc(
                    sems["outst"], 16)
        for k in range(NCH - 3, NCH):
            sync.wait_ge(sems["v_done"], k + 1)
            sync.dma_start(out=o_d[k], in_=ov(k)).then_inc(sems["outst"], 16)

    @blk.vector
    def _(vector):
        tt = nc.vector.tensor_tensor

        def cb_(i):
            return cst_sb[:, i:i + 1].to_broadcast([P, K])

        vector.wait_ge(sems["c_sem"], 16)
        for j in range(NCH + 3):
            if j < NCH:
                H, T, R = hv(j), tv(j), rv(j)
                NS, PP = nsv(j), ppv(j)
                vector.wait_ge(sems["in_sem"], 48 * (j + 1))
                # rotation (C,S pre-scaled by cvn)
                tt(out=PP[:, :, :], in0=R[:, :, 0:32], in1=H[:, :, 1:33],
                   op=MUL)
                tt(out=NS[:, :, 0:16], in0=PP[:, :, 0:16],
                   in1=PP[:, :, 16:32], op=SUB)
                tt(out=PP[:, :, 0:16], in0=R[:, :, 16:32], in1=H[:, :, 1:17],
                   op=MUL)
                tt(out=PP[:, :, 16:32], in0=R[:, :, 0:16], in1=H[:, :, 17:33],
                   op=MUL)
                tt(out=NS[:, :, 16:32], in0=PP[:, :, 0:16],
                   in1=PP[:, :, 16:32], op=ADD)
                # boost: ns0 = c0*(cvn*ra0) + x0*(cvn*s0)
                tt(out=sm("xs", j), in0=H[:, :, 0], in1=R[:, :, 64], op=MUL)
                tt(out=sm("t0c", j), in0=NS[:, :, 0], in1=R[:, :, 65], op=MUL)
                tt(out=NS[:, :, 0], in0=sm("t0c", j), in1=sm("xs", j), op=ADD)
                # res = ns + w
                tt(out=NS[:, :, :], in0=NS[:, :, :], in1=R[:, :, 32:64],
                   op=ADD)
                vector.drain()
                vector.sem_inc(sems["v_rs"], 1)
                # PD = res * t_sp, plus early extraction of t0 and bh+bt
                tt(out=PP[:, :, :], in0=NS[:, :, :], in1=T[:, :, 1:33],
                   op=MUL)
                tt(out=sm("tt0", j), in0=T[:, :, 0], in1=cb_(4), op=MUL)
                tt(out=sm("b1", j), in0=H[:, :, 33], in1=T[:, :, 33], op=ADD)
                vector.drain()
                vector.sem_inc(sems["v_big"], 1)
                vector.wait_ge(sems["a_sq"], j + 1)
                nc.vector.reduce_sum(out=sm("r2", j), in_=sqv(j)[:, :, :],
                                     axis=mybir.AxisListType.X)
                nc.vector.reduce_sum(out=sm("dot", j), in_=PP[:, :, :],
                                     axis=mybir.AxisListType.X)
            ja = j - 1
            if 0 <= ja < NCH:
                # tail A: time = sqrt(1+r2) (deg-4 poly, r2<=0.02), ic, qm
                x = sm("r2", ja)
                tt(out=sm("xs", ja), in0=x, in1=cb_(0), op=MUL)
                tt(out=sm("xs", ja), in0=sm("xs", ja), in1=cb_(1), op=ADD)
                tt(out=sm("t0c", ja), in0=sm("xs", ja), in1=x, op=MUL)
                tt(out=sm("t0c", ja), in0=sm("t0c", ja), in1=cb_(2), op=ADD)
                tt(out=sm("xs", ja), in0=sm("t0c", ja), in1=x, op=MUL)
                tt(out=sm("xs", ja), in0=sm("xs", ja), in1=cb_(3), op=ADD)
                tt(out=sm("time", ja), in0=sm("xs", ja), in1=x, op=MUL)
                tt(out=sm("time", ja), in0=sm("time", ja), in1=cb_(4), op=ADD)
                tt(out=sm("u1", ja), in0=sm("time", ja), in1=sm("tt0", ja),
                   op=MUL)
                tt(out=sm("u1", ja), in0=sm("u1", ja), in1=sm("dot", ja),
                   op=SUB)
                tt(out=sm("ic", ja), in0=sm("u1", ja), in1=cb_(5), op=MAX)
                tt(out=sm("xs", ja), in0=sm("ic", ja), in1=cb_(4), op=SUB)
                tt(out=sm("t0c", ja), in0=sm("ic", ja), in1=cb_(4), op=ADD)
                tt(out=sm("qm", ja), in0=sm("xs", ja), in1=sm("t0c", ja),
                   op=MUL)
                vector.drain()
                vector.sem_inc(sems["v_ic"], 1)
            jb = j - 2
            if 0 <= jb < NCH:
                vector.wait_ge(sems["a_s2"], jb + 1)
                tt(out=sm("u", jb), in0=sm("ic", jb), in1=sm("s2", jb),
                   op=ADD)
                vector.drain()
                vector.sem_inc(sems["v_u"], 1)
            jc = j - 3
            if 0 <= jc < NCH:
                if jc >= 2:
                    vector.wait_ge(sems["outst"], 16 * (jc - 1))
                vector.wait_ge(sems["a_dd"], jc + 1)
                if DBG_OUT is None:
                    tt(out=ov(jc), in0=sm("b1", jc), in1=sm("dd", jc), op=SUB)
                else:
                    nc.vector.tensor_copy(out=ov(jc), in_=sm(DBG_OUT, jc))
                vector.drain()
                vector.sem_inc(sems["v_done"], 1)

    @blk.scalar
    def _(scalar):
        act = nc.scalar.activation
        AF = mybir.ActivationFunctionType
        for j in range(NCH + 2):
            if j < NCH:
                scalar.wait_ge(sems["v_rs"], j + 1)
                act(out=sqv(j)[:, :, :], in_=nsv(j)[:, :, :], func=AF.Square)
                scalar.drain()
                scalar.sem_inc(sems["a_sq"], 1)
            ja = j - 1
            if 0 <= ja < NCH:
                scalar.wait_ge(sems["v_ic"], ja + 1)
                act(out=sm("s2", ja), in_=sm("qm", ja), func=AF.Sqrt)
                scalar.drain()
                scalar.sem_inc(sems["a_s2"], 1)
            jb = j - 2
            if 0 <= jb < NCH:
                scalar.wait_ge(sems["v_u"], jb + 1)
                act(out=sm("dd", jb), in_=sm("u", jb), func=AF.Ln)
                act(out=sm("dd", jb), in_=sm("dd", jb), func=AF.Square)
                scalar.drain()
                scalar.sem_inc(sems["a_dd"], 1)

    blk_cm.__exit__(None, None, None)
    nc._ctx_keepalive = ctx_list
    return nc


def _get_nc():
    if not _NC_CACHE:
        _NC_CACHE.append(_build_nc())
    return _NC_CACHE[0]


def _host_pack(heads, relations, tails, entity_emb, rel_boost_w, rel_rot_w,
               rel_trans_w, ent_bias_w):
    heads = np.asarray(heads).astype(np.int64)
    relations = np.asarray(relations).astype(np.int64)
    tails = np.asarray(tails).astype(np.int64)
    entity_emb = np.asarray(entity_emb, dtype=np.float32)
    ent_bias_w = np.asarray(ent_bias_w, dtype=np.float32)

    rot = np.asarray(rel_rot_w, dtype=np.float32).astype(np.float64)
    boost = np.asarray(rel_boost_w, dtype=np.float32).astype(np.float64)
    trans = np.asarray(rel_trans_w, dtype=np.float32).astype(np.float64)

    c = np.cos(rot[:, :16])
    s = np.sin(rot[:, :16])
    rap0 = np.clip(boost[:, 0], -2.0, 2.0)
    c0 = np.cosh(rap0)
    s0 = np.sinh(rap0)
    tv = 0.1 * trans
    vn = np.sqrt(np.clip(np.sum(tv * tv, axis=1), 1e-6, None))
    cvn = np.cosh(vn)
    w = (np.sinh(vn) / vn)[:, None] * tv

    rel_packed = np.zeros((NR, RW), dtype=np.float32)
    rel_packed[:, 0:16] = (cvn[:, None] * c).astype(np.float32)
    rel_packed[:, 16:32] = (cvn[:, None] * s).astype(np.float32)
    rel_packed[:, 32:64] = w.astype(np.float32)
    rel_packed[:, 64] = (cvn * s0).astype(np.float32)
    rel_packed[:, 65] = c0.astype(np.float32)

    ent_packed = np.concatenate([entity_emb, ent_bias_w], axis=1)  # [NE, 34]

    h_stream = ent_packed[heads]
    t_stream = ent_packed[tails]
    r_stream = rel_packed[relations]
    return h_stream, t_stream, r_stream


def kernel(heads, relations, tails, entity_emb, rel_boost_w, rel_rot_w,
           rel_trans_w, ent_bias_w):
    global LAST_EXEC_NS
    h_stream, t_stream, r_stream = _host_pack(
        heads, relations, tails, entity_emb, rel_boost_w, rel_rot_w,
        rel_trans_w, ent_bias_w)

    nc = _get_nc()
    cst = np.zeros((P, 8), dtype=np.float32)
    cst[:, 0] = -0.0390625
    cst[:, 1] = 0.0625
    cst[:, 2] = -0.125
    cst[:, 3] = 0.5
    cst[:, 4] = 1.0
    cst[:, 5] = 1.0 + 1e-6
    in_maps = []
    for i in range(NCORES):
        sl = slice(i * BCORE, (i + 1) * BCORE)
        in_maps.append({"h": np.ascontiguousarray(h_stream[sl]),
                        "t": np.ascontiguousarray(t_stream[sl]),
                        "r": np.ascontiguousarray(r_stream[sl]),
                        "cst": cst})

    res = run_bass_kernel_spmd(nc, in_maps, core_ids=list(range(NCORES)),
                               trace=TRACE)
    LAST_EXEC_NS = res.exec_time_ns
    return np.concatenate([res.results[i]["out"] for i in range(NCORES)])


# revision 17
# speedup vs baseline: 1.5566x; 1.1068x over previous
"""LorentzKG scoring kernel for 8 Trainium2 NeuronCores. v3: 5-stage softpipe,
PD multiply on GPSIMD (full-iteration slack), merged drains.

Engine schedule (DVE iteration j):
  big(j):      rotation/boost/res -> one drain -> v_rs + v_big
  reduce(j-1): r2/dot (after ACT square a_sq(j-1) and GPSIMD pd g_pd(j-1))
  tailA(j-2):  sqrt-poly, ic, qm   -> v_ic
  tailB1(j-3): u = ic + s2         -> v_u
  tailB2(j-4): score = b1 - dd     -> v_done
ACT iter j: square(j); s2(j-2); ln+dd(j-3).
GPSIMD iter j: pd(j) = res * t_sp (after v_rs(j)).
sync: prefetch j+2 after v_big(j) & g_pd(j); store j-4 after v_done.
"""
import numpy as np

import concourse.bass as bass
import concourse.mybir as mybir
from concourse.bass_utils import run_bass_kernel_spmd

NE = 1_000_000
NR = 1000
D = 32
B = 1_048_576
NCORES = 8
BCORE = B // NCORES          # 131072
P = 128
K = 64                       # triples per partition per chunk
CHUNK = P * K                # 8192
NCH = BCORE // CHUNK         # 16
HW = 34
RW = 68

TRACE = False
LAST_EXEC_NS = None
DBG_OUT = None

_NC_CACHE = []

F32 = mybir.dt.float32
MUL = mybir.AluOpType.mult
ADD = mybir.AluOpType.add
SUB = mybir.AluOpType.subtract
MAX = mybir.AluOpType.max

SM4 = ["tt0", "b1", "r2", "dot", "time", "xs", "t0c", "u1", "ic", "qm",
       "s2", "u", "dd"]


def _build_nc():
    nc = bass.Bass()
    h_in = nc.declare_dram_parameter("h", [BCORE, HW], F32, isOutput=False)
    t_in = nc.declare_dram_parameter("t", [BCORE, HW], F32, isOutput=False)
    r_in = nc.declare_dram_parameter("r", [BCORE, RW], F32, isOutput=False)
    cst_in = nc.declare_dram_parameter("cst", [P, 8], F32, isOutput=False)
    out = nc.declare_dram_parameter("out", [BCORE], F32, isOutput=True)

    h_d = h_in[:].rearrange("(c p k) d -> c p (k d)", p=P, k=K)
    t_d = t_in[:].rearrange("(c p k) d -> c p (k d)", p=P, k=K)
    r_d = r_in[:].rearrange("(c p k) d -> c p (k d)", p=P, k=K)
    o_d = out[:].rearrange("(c p k) -> c p k", p=P, k=K)

    ctx_list = []

    def sb(width):
        cm = nc.sbuf_tensor([P, width], F32)
        t = cm.__enter__()
        ctx_list.append(cm)
        return t

    cst_sb = sb(8)
    h_sb = sb(2 * K * HW)
    t_sb = sb(2 * K * HW)
    r_sb = sb(2 * K * RW)
    ns_sb = sb(2 * K * 32)
    sq_sb = sb(2 * K * 32)
    pp_sb = sb(2 * K * 32)       # P1 during rotation, then PD (written by gpsimd)
    o_sb = sb(2 * K)
    smalls = {n: sb(4 * K) for n in SM4}
    smalls["b1"] = sb(8 * K)

    sems = {}
    for n in ["in_sem", "outst", "c_sem", "v_big", "v_rs", "v_ic", "v_u",
              "v_done", "a_sq", "a_s2", "a_dd", "g_pd"]:
        cm = nc.semaphore(n)
        sems[n] = cm.__enter__()
        ctx_list.append(cm)

    def view(t, j, width, d):
        s = j % 2
        return t[:, s * K * width:(s + 1) * K * width].rearrange(
            "p (k d) -> p k d", d=d)

    def hv(j):
        return view(h_sb, j, HW, HW)

    def tv(j):
        return view(t_sb, j, HW, HW)

    def rv(j):
        return view(r_sb, j, RW, RW)

    def nsv(j):
        return view(ns_sb, j, 32, 32)

    def sqv(j):
        return view(sq_sb, j, 32, 32)

    def ppv(j):
        return view(pp_sb, j, 32, 32)

    def ov(j):
        s = j % 2
        return o_sb[:, s * K:(s + 1) * K]

    def sm(n, j):
        s = j % (8 if n == "b1" else 4)
        return smalls[n][:, s * K:(s + 1) * K]

    blk_cm = nc.Block()
    blk = blk_cm.__enter__()

    @blk.sync
    def _(sync):
        sync.dma_start(out=cst_sb[:, 0:8], in_=cst_in[:]).then_inc(
            sems["c_sem"], 16)
        for j in range(min(2, NCH)):
            sync.dma_start(out=hv(j), in_=h_d[j]).then_inc(sems["in_sem"], 16)
            sync.dma_start(out=tv(j), in_=t_d[j]).then_inc(sems["in_sem"], 16)
            sync.dma_start(out=rv(j), in_=r_d[j]).then_inc(sems["in_sem"], 16)
        for j in range(NCH):
            if j + 2 < NCH:
                sync.wait_ge(sems["v_big"], j + 1)
                sync.wait_ge(sems["g_pd"], j + 1)
                sync.dma_start(out=hv(j + 2), in_=h_d[j + 2]).then_inc(
                    sems["in_sem"], 16)
                sync.dma_start(out=tv(j + 2), in_=t_d[j + 2]).then_inc(
                    sems["in_sem"], 16)
                sync.dma_start(out=rv(j + 2), in_=r_d[j + 2]).then_inc(
                    sems["in_sem"], 16)
            if j >= 4:
                sync.wait_ge(sems["v_done"], j - 3)
                sync.dma_start(out=o_d[j - 4], in_=ov(j - 4)).then_inc(
                    sems["outst"], 16)
        for k in range(max(NCH - 4, 0), NCH):
            sync.wait_ge(sems["v_done"], k + 1)
            sync.dma_start(out=o_d[k], in_=ov(k)).then_inc(sems["outst"], 16)

    @blk.vector
    def _(vector):
        tt = nc.vector.tensor_tensor

        def cb_(i):
            return cst_sb[:, i:i + 1].to_broadcast([P, K])

        vector.wait_ge(sems["c_sem"], 16)
        for j in range(NCH + 4):
            if j < NCH:
                H, T, R = hv(j), tv(j), rv(j)
                NS, PP = nsv(j), ppv(j)
                vector.wait_ge(sems["in_sem"], 48 * (j + 1))
                tt(out=PP[:, :, :], in0=R[:, :, 0:32], in1=H[:, :, 1:33],
                   op=MUL)
                tt(out=NS[:, :, 0:16], in0=PP[:, :, 0:16],
                   in1=PP[:, :, 16:32], op=SUB)
                tt(out=PP[:, :, 0:16], in0=R[:, :, 16:32], in1=H[:, :, 1:17],
                   op=MUL)
                tt(out=PP[:, :, 16:32], in0=R[:, :, 0:16], in1=H[:, :, 17:33],
                   op=MUL)
                tt(out=NS[:, :, 16:32], in0=PP[:, :, 0:16],
                   in1=PP[:, :, 16:32], op=ADD)
                tt(out=sm("xs", j), in0=H[:, :, 0], in1=R[:, :, 64], op=MUL)
                tt(out=sm("t0c", j), in0=NS[:, :, 0], in1=R[:, :, 65], op=MUL)
                tt(out=NS[:, :, 0], in0=sm("t0c", j), in1=sm("xs", j), op=ADD)
                tt(out=NS[:, :, :], in0=NS[:, :, :], in1=R[:, :, 32:64],
                   op=ADD)
                tt(out=sm("tt0", j), in0=T[:, :, 0], in1=cb_(4), op=MUL)
                tt(out=sm("b1", j), in0=H[:, :, 33], in1=T[:, :, 33], op=ADD)
                vector.drain()
                vector.sem_inc(sems["v_rs"], 1)
                vector.sem_inc(sems["v_big"], 1)
            jr = j - 1
            if 0 <= jr < NCH:
                vector.wait_ge(sems["a_sq"], jr + 1)
                vector.wait_ge(sems["g_pd"], jr + 1)
                nc.vector.reduce_sum(out=sm("r2", jr), in_=sqv(jr)[:, :, :],
                                     axis=mybir.AxisListType.X)
                nc.vector.reduce_sum(out=sm("dot", jr), in_=ppv(jr)[:, :, :],
                                     axis=mybir.AxisListType.X)
            ja = j - 2
            if 0 <= ja < NCH:
                x = sm("r2", ja)
                tt(out=sm("xs", ja), in0=x, in1=cb_(0), op=MUL)
                tt(out=sm("xs", ja), in0=sm("xs", ja), in1=cb_(1), op=ADD)
                tt(out=sm("t0c", ja), in0=sm("xs", ja), in1=x, op=MUL)
                tt(out=sm("t0c", ja), in0=sm("t0c", ja), in1=cb_(2), op=ADD)
                tt(out=sm("xs", ja), in0=sm("t0c", ja), in1=x, op=MUL)
                tt(out=sm("xs", ja), in0=sm("xs", ja), in1=cb_(3), op=ADD)
                tt(out=sm("time", ja), in0=sm("xs", ja), in1=x, op=MUL)
                tt(out=sm("time", ja), in0=sm("time", ja), in1=cb_(4), op=ADD)
                tt(out=sm("u1", ja), in0=sm("time", ja), in1=sm("tt0", ja),
                   op=MUL)
                tt(out=sm("u1", ja), in0=sm("u1", ja), in1=sm("dot", ja),
                   op=SUB)
                tt(out=sm("ic", ja), in0=sm("u1", ja), in1=cb_(5), op=MAX)
                tt(out=sm("xs", ja), in0=sm("ic", ja), in1=cb_(4), op=SUB)
                tt(out=sm("t0c", ja), in0=sm("ic", ja), in1=cb_(4), op=ADD)
                tt(out=sm("qm", ja), in0=sm("xs", ja), in1=sm("t0c", ja),
                   op=MUL)
                vector.drain()
                vector.sem_inc(sems["v_ic"], 1)
            jb = j - 3
            if 0 <= jb < NCH:
                vector.wait_ge(sems["a_s2"], jb + 1)
                tt(out=sm("u", jb), in0=sm("ic", jb), in1=sm("s2", jb),
                   op=ADD)
                vector.drain()
                vector.sem_inc(sems["v_u"], 1)
            jc = j - 4
            if 0 <= jc < NCH:
                if jc >= 2:
                    vector.wait_ge(sems["outst"], 16 * (jc - 1))
                vector.wait_ge(sems["a_dd"], jc + 1)
                if DBG_OUT is None:
                    tt(out=ov(jc), in0=sm("b1", jc), in1=sm("dd", jc), op=SUB)
                else:
                    nc.vector.tensor_copy(out=ov(jc), in_=sm(DBG_OUT, jc))
                vector.drain()
                vector.sem_inc(sems["v_done"], 1)

    @blk.scalar
    def _(scalar):
        act = nc.scalar.activation
        AF = mybir.ActivationFunctionType
        for j in range(NCH + 3):
            if j < NCH:
                scalar.wait_ge(sems["v_rs"], j + 1)
                act(out=sqv(j)[:, :, :], in_=nsv(j)[:, :, :], func=AF.Square)
                scalar.drain()
                scalar.sem_inc(sems["a_sq"], 1)
            ja = j - 2
            if 0 <= ja < NCH:
                scalar.wait_ge(sems["v_ic"], ja + 1)
                act(out=sm("s2", ja), in_=sm("qm", ja), func=AF.Sqrt)
                scalar.drain()
                scalar.sem_inc(sems["a_s2"], 1)
            jb = j - 3
            if 0 <= jb < NCH:
                scalar.wait_ge(sems["v_u"], jb + 1)
                act(out=sm("dd", jb), in_=sm("u", jb), func=AF.Ln)
                act(out=sm("dd", jb), in_=sm("dd", jb), func=AF.Square)
                scalar.drain()
                scalar.sem_inc(sems["a_dd"], 1)

    @blk.gpsimd
    def _(gpsimd):
        for j in range(NCH):
            gpsimd.wait_ge(sems["v_rs"], j + 1)
            nc.gpsimd.tensor_tensor(
                out=ppv(j)[:, :, :], in0=nsv(j)[:, :, :],
                in1=tv(j)[:, :, 1:33], op=MUL)
            gpsimd.drain()
            gpsimd.sem_inc(sems["g_pd"], 1)

    blk_cm.__exit__(None, None, None)
    nc._ctx_keepalive = ctx_list
    return nc


def _get_nc():
    if not _NC_CACHE:
        _NC_CACHE.append(_build_nc())
    return _NC_CACHE[0]


def _host_pack(heads, relations, tails, entity_emb, rel_boost_w, rel_rot_w,
               rel_trans_w, ent_bias_w):
    heads = np.asarray(heads).astype(np.int64)
    relations = np.asarray(relations).astype(np.int64)
    tails = np.asarray(tails).astype(np.int64)
    entity_emb = np.asarray(entity_emb, dtype=np.float32)
    ent_bias_w = np.asarray(ent_bias_w, dtype=np.float32)

    rot = np.asarray(rel_rot_w, dtype=np.float32).astype(np.float64)
    boost = np.asarray(rel_boost_w, dtype=np.float32).astype(np.float64)
    trans = np.asarray(rel_trans_w, dtype=np.float32).astype(np.float64)

    c = np.cos(rot[:, :16])
    s = np.sin(rot[:, :16])
    rap0 = np.clip(boost[:, 0], -2.0, 2.0)
    c0 = np.cosh(rap0)
    s0 = np.sinh(rap0)
    tv = 0.1 * trans
    vn = np.sqrt(np.clip(np.sum(tv * tv, axis=1), 1e-6, None))
    cvn = np.cosh(vn)
    w = (np.sinh(vn) / vn)[:, None] * tv

    rel_packed = np.zeros((NR, RW), dtype=np.float32)
    rel_packed[:, 0:16] = (cvn[:, None] * c).astype(np.float32)
    rel_packed[:, 16:32] = (cvn[:, None] * s).astype(np.float32)
    rel_packed[:, 32:64] = w.astype(np.float32)
    rel_packed[:, 64] = (cvn * s0).astype(np.float32)
    rel_packed[:, 65] = c0.astype(np.float32)

    ent_packed = np.concatenate([entity_emb, ent_bias_w], axis=1)

    h_stream = ent_packed[heads]
    t_stream = ent_packed[tails]
    r_stream = rel_packed[relations]
    return h_stream, t_stream, r_stream


def kernel(heads, relations, tails, entity_emb, rel_boost_w, rel_rot_w,
           rel_trans_w, ent_bias_w):
    global LAST_EXEC_NS
    h_stream, t_stream, r_stream = _host_pack(
        heads, relations, tails, entity_emb, rel_boost_w, rel_rot_w,
        rel_trans_w, ent_bias_w)

    nc = _get_nc()
    cst = np.zeros((P, 8), dtype=np.float32)
    cst[:, 0] = -0.0390625
    cst[:, 1] = 0.0625
    cst[:, 2] = -0.125
    cst[:, 3] = 0.5
    cst[:, 4] = 1.0
    cst[:, 5] = 1.0 + 1e-6
    in_maps = []
    for i in range(NCORES):
        sl = slice(i * BCORE, (i + 1) * BCORE)
        in_maps.append({"h": np.ascontiguousarray(h_stream[sl]),
                        "t": np.ascontiguousarray(t_stream[sl]),
                        "r": np.ascontiguousarray(r_stream[sl]),
                        "cst": cst})

    res = run_bass_kernel_spmd(nc, in_maps, core_ids=list(range(NCORES)),
                               trace=TRACE)
    LAST_EXEC_NS = res.exec_time_ns
    return np.concatenate([res.results[i]["out"] for i in range(NCORES)])
